# revision 21
# baseline (speedup 1.0000x reference)
"""GAT link-prediction kernel for Trainium2, 8-core SPMD (v2, fp16).

Strategy (graph/data parallel per the dst-owner sharding hint):
- Nodes relabeled: discrepancy-colored into two table halves (balances
  every dst's source split, tightening ELL maxima), dealt to 8 cores by
  degree rank, snake-packed into 49 dst tiles of 128 per core.
- Per layer: node phase computes [h@W | h@a_s | h@a_d] fp16 per shard
  (edge output stays in SBUF and feeds the next layer's node matmul);
  the table is replicated via AllGather (sim build: per-tile 8-fold
  strided DMA copies so replication pipelines behind the node phase).
- Edge phase per 128-dst tile: two dma_gathers (int16 indices, two
  25088-row halves, 512B fp16 rows), per-half score pipeline (fp16 DVE
  ops + one exp per half with fp32 accumulated softmax sums), then
  per-slot aggregation: DVE tensor_scalar builds diag(p)*I fp16 (4x
  mode) and the PE matmul-accumulates into PSUM fp32. Self-loop edges
  never leave the core: their slot reads the local projection tile.
- GCN layer: same ELL with precomputed fp32 norm weights; z fp16.
- Decode: label edges grouped by (src-half, dst-half) in variable-size
  batches; two dma_gathers per batch, fp16 dot products reduced on the
  free axis into fp32 logits.
"""
import numpy as np
from concourse import bass, bacc, mybir, tile, bass_utils

NCORES = 8
N = 50000
IN = 128
HID = 128
OUT = 64
NL = 200000
NEG = 0.2

SP = 6272                 # padded nodes per core (49 * 128)
G = NCORES * SP           # 50176 padded global nodes
HALF = G // 2             # 25088 (< int16 max)
NT = SP // 128            # 49 dst tiles per core
WG = 256                  # GAT table row fp16: 128 h|1 hs|1 hd|126 pad (512B)
WZ = 128                  # z table row fp16: 64 z|64 pad (256B)
PB = 2048                 # decode gather batch (indices)
PBC = PB // 128           # 16 label-tile chunks per batch
WO = 132                  # SBUF-resident ot width: 128 h|1 hs|1 hd|2 pad

f32 = mybir.dt.float32
f16 = mybir.dt.float16
i16 = mybir.dt.int16


def _wrap16(flat):
    """dma_gather index layout: value at [j%16, j//16], replicated to all
    8 gpsimd core groups -> [128, n//16] int16."""
    n = len(flat)
    cols = n // 16
    blk = np.ascontiguousarray(flat.astype(np.int16).reshape(cols, 16).T)
    return np.tile(blk, (8, 1))


def _prep(x, ei, eli, W1, a1s, a1d, b1, W2, a2s, a2d, b2,
          W3, a3s, a3d, b3, W4, b4):
    src = np.asarray(ei[0], np.int64)
    dst = np.asarray(ei[1], np.int64)

    deg = np.bincount(dst, minlength=N) + 1          # with self-loop (norm)
    order = np.argsort(-deg, kind="stable")
    # group (= table half) assignment: start with degree-rank parity, then
    # discrepancy passes balancing every dst's source split c0~c1 --
    # tightens the per-tile ELL maxima K0+K1 (fewer padded gather slots)
    grp = np.empty(N, np.int8)
    grp[order] = (np.arange(N) % NCORES >= NCORES // 2).astype(np.int8)
    outdeg = np.bincount(src, minlength=N).astype(np.float64)
    for cap in (3000, 1500, 800, 500, 350, 250, 180, 130, 90, 60):
        c0n = np.bincount(dst[grp[src] == 0], minlength=N)
        c1n = np.bincount(dst[grp[src] == 1], minlength=N)
        imb = (c0n - c1n).astype(np.float64)
        s_u = np.bincount(src, weights=imb[dst], minlength=N)
        sgn = 1.0 - 2.0 * grp
        gain = -4.0 * sgn * s_u + 4.0 * outdeg
        i0 = np.where((grp == 0) & (gain < 0))[0]
        i1 = np.where((grp == 1) & (gain < 0))[0]
        i0 = i0[np.argsort(gain[i0])]
        i1 = i1[np.argsort(gain[i1])]
        k = min(len(i0), len(i1), cap)
        if k == 0:
            break
        grp[i0[:k]] = 1
        grp[i1[:k]] = 0
    c0n = np.bincount(dst[grp[src] == 0], minlength=N)
    c1n = np.bincount(dst[grp[src] == 1], minlength=N)
    # cores: deal by degree rank within each group; group0 -> cores 0..3
    core = np.empty(N, np.int64)
    g0 = order[grp[order] == 0]
    g1 = order[grp[order] == 1]
    core[g0] = np.arange(len(g0)) % (NCORES // 2)
    core[g1] = NCORES // 2 + np.arange(len(g1)) % (NCORES // 2)
    # within-core snake order: c0 desc, then c1 desc inside 768-blocks
    newid = np.empty(N, np.int64)
    for c in range(NCORES):
        nodes = np.where(core == c)[0]
        o = nodes[np.lexsort((-c1n[nodes], -c0n[nodes]))]
        parts = []
        for i in range(0, len(o), 768):
            blk = o[i:i + 768]
            parts.append(blk[np.argsort(-c1n[blk], kind="stable")])
        o = np.concatenate(parts)
        newid[o] = c * SP + np.arange(len(o))

    S = newid[src]
    D = newid[dst]
    ne = S.shape[0]

    deg_g = np.zeros(G, np.int64)
    deg_g[newid] = deg
    dinv = np.zeros(G, np.float64)
    nz = deg_g > 0
    dinv[nz] = 1.0 / np.sqrt(deg_g[nz])

    half = (S >= HALF).astype(np.int64)
    loc16 = S - half * HALF
    key = D * 2 + half
    sidx = np.argsort(key, kind="stable")
    ks = key[sidx]
    Ss = S[sidx]
    loc_s = loc16[sidx]
    cnt = np.bincount(key, minlength=2 * G)
    startp = np.zeros(2 * G + 1, np.int64)
    np.cumsum(cnt, out=startp[1:])
    slot = np.arange(ne, dtype=np.int64) - startp[ks]

    c0 = cnt[0::2].reshape(NCORES, NT, 128)
    c1 = cnt[1::2].reshape(NCORES, NT, 128)
    K0 = np.maximum(c0.max(axis=(0, 2)), 1).astype(int)
    K1 = np.maximum(c1.max(axis=(0, 2)), 1).astype(int)
    K0m, K1m = int(K0.max()), int(K1.max())

    e0 = (ks % 2) == 0
    e1 = ~e0
    d_all = ks // 2
    grid0 = np.zeros((G, K0m), np.int16)
    vm0 = np.zeros((G, K0m), bool)
    grid0[d_all[e0], slot[e0]] = loc_s[e0].astype(np.int16)
    vm0[d_all[e0], slot[e0]] = True
    grid1 = np.zeros((G, K1m), np.int16)
    vm1 = np.zeros((G, K1m), bool)
    grid1[d_all[e1], slot[e1]] = loc_s[e1].astype(np.int16)
    vm1[d_all[e1], slot[e1]] = True
    nval = (dinv[Ss] * dinv[d_all]).astype(np.float32)
    nw0 = np.zeros((G, K0m), np.float32)
    nw0[d_all[e0], slot[e0]] = nval[e0]
    nw1 = np.zeros((G, K1m), np.float32)
    nw1[d_all[e1], slot[e1]] = nval[e1]

    # permuted node features fp16, padded
    x = np.asarray(x, np.float32)
    xg = np.zeros((G, IN), np.float16)
    xg[newid] = x.astype(np.float16)

    # packed weights fp16
    def pack(W, as_, ad_):
        W = np.asarray(W, np.float32)
        out = np.zeros((IN, WO), np.float32)
        out[:, :HID] = W
        out[:, HID] = W @ np.asarray(as_, np.float32)
        out[:, HID + 1] = W @ np.asarray(ad_, np.float32)
        return out.astype(np.float16)
    wx = [pack(W1, a1s, a1d), pack(W2, a2s, a2d), pack(W3, a3s, a3d)]
    w4 = np.asarray(W4, np.float32).astype(np.float16)
    bias = [np.asarray(b, np.float32).astype(np.float16).reshape(1, -1)
            for b in (b1, b2, b3, b4)]

    # decode: shard label edges by position, group by (halfA, halfB)
    A = newid[np.asarray(eli[0], np.int64)]
    B = newid[np.asarray(eli[1], np.int64)]
    npc = NL // NCORES
    gidx = [(A[c * npc:(c + 1) * npc] >= HALF) * 2 +
            (B[c * npc:(c + 1) * npc] >= HALF) for c in range(NCORES)]
    gcounts = np.array([np.bincount(g, minlength=4) for g in gidx])
    gpad = [int(-(-gcounts[:, g].max() // 128)) * 128 for g in range(4)]
    DB = []                                     # (group, batch_size) list
    for g in range(4):
        rem = gpad[g]
        while rem > 0:
            s = min(rem, PB)
            DB.append((g, s))
            rem -= s
    TOTC = sum(s // 128 for _, s in DB)

    off0 = np.concatenate([[0], np.cumsum(K0)])
    off1 = np.concatenate([[0], np.cumsum(K1)])
    offm = np.concatenate([[0], np.cumsum(K0 + K1)])
    SK0, SK1, SKT = int(off0[-1]), int(off1[-1]), int(offm[-1])

    in_maps = []
    unshard = []
    for c in range(NCORES):
        rows = slice(c * SP, (c + 1) * SP)
        ix0p, ix1p, mkp, nwp = [], [], [], []
        for t in range(NT):
            r = slice(c * SP + t * 128, c * SP + (t + 1) * 128)
            k0, k1 = K0[t], K1[t]
            f0 = np.ascontiguousarray(grid0[r, :k0].T).reshape(-1)
            f1 = np.ascontiguousarray(grid1[r, :k1].T).reshape(-1)
            ix0p.append(_wrap16(f0))                 # [128, 8*k0]
            ix1p.append(_wrap16(f1))
            m = np.full((128, k0 + k1), -30000.0, np.float16)
            m[:, :k0][vm0[r, :k0]] = 0.0
            m[:, k0:][vm1[r, :k1]] = 0.0
            mkp.append(m)
            w = np.concatenate([nw0[r, :k0], nw1[r, :k1]], axis=1)
            nwp.append(np.ascontiguousarray(w))

        dv2 = (dinv[c * SP:(c + 1) * SP] ** 2).astype(np.float32)
        dv2 = np.ascontiguousarray(dv2.reshape(NT, 128).T)   # [128, NT]

        Ac, Bc = A[c * npc:(c + 1) * npc], B[c * npc:(c + 1) * npc]
        gc = gidx[c]
        ordc = np.argsort(gc, kind="stable")
        diap, dibp = [], []
        for g in range(4):
            sel = ordc[gc[ordc] == g]
            na = gpad[g]
            av = np.zeros(na, np.int64)
            bv = np.zeros(na, np.int64)
            av[:len(sel)] = Ac[sel] - (g >> 1) * HALF
            bv[:len(sel)] = Bc[sel] - (g & 1) * HALF
            o = 0
            for gg, s in DB:
                if gg != g:
                    continue
                diap.append(_wrap16(av[o:o + s]))     # [128, s//16]
                dibp.append(_wrap16(bv[o:o + s]))
                o += s

        im = {
            "xs": np.ascontiguousarray(xg[rows]),
            "ix0": np.concatenate(ix0p, axis=1),     # [128, 8*SK0] i16
            "ix1": np.concatenate(ix1p, axis=1),
            "msk": np.concatenate(mkp, axis=1),      # [128, SKT] f16
            "nwt": np.concatenate(nwp, axis=1),
            "dv2": dv2,                              # [128, NT] f32
            "dia": np.concatenate(diap, axis=1),     # [128, TOTB*128] i16
            "dib": np.concatenate(dibp, axis=1),
            "wx1": wx[0], "wx2": wx[1], "wx3": wx[2], "w4p": w4,
            "bi1": bias[0], "bi2": bias[1], "bi3": bias[2], "bi4": bias[3],
        }
        in_maps.append(im)
        unshard.append(ordc)

    prof = {
        "K0": [int(v) for v in K0], "K1": [int(v) for v in K1],
        "DB": DB, "TOTC": TOTC,
        "SK0": SK0, "SK1": SK1, "SKT": SKT,
    }
    meta = {"gcounts": gcounts, "npc": npc}
    return prof, in_maps, unshard, meta


def _build(prof, sim_mode=False):
    K0, K1 = prof["K0"], prof["K1"]
    DB, TOTC = prof["DB"], prof["TOTC"]
    SK0, SK1, SKT = prof["SK0"], prof["SK1"], prof["SKT"]
    AluOp = mybir.AluOpType
    Act = mybir.ActivationFunctionType

    nc = bacc.Bacc("TRN2", target_bir_lowering=False, debug=False,
                   num_devices=NCORES, dynamic_dma_scratch_size=65536)

    xs = nc.dram_tensor("xs", [SP, IN], f16, kind="ExternalInput")
    wxh = [nc.dram_tensor(f"wx{l}", [IN, WO], f16, kind="ExternalInput")
           for l in (1, 2, 3)]
    w4h = nc.dram_tensor("w4p", [HID, OUT], f16, kind="ExternalInput")
    bih = [nc.dram_tensor(f"bi{l}", [1, HID if l < 4 else OUT], f16,
                          kind="ExternalInput") for l in (1, 2, 3, 4)]
    ix0h = nc.dram_tensor("ix0", [128, 8 * SK0], i16, kind="ExternalInput")
    ix1h = nc.dram_tensor("ix1", [128, 8 * SK1], i16, kind="ExternalInput")
    mskh = nc.dram_tensor("msk", [128, SKT], f16, kind="ExternalInput")
    nwth = nc.dram_tensor("nwt", [128, SKT], f32, kind="ExternalInput")
    dv2h = nc.dram_tensor("dv2", [128, NT], f32, kind="ExternalInput")
    diah = nc.dram_tensor("dia", [128, 8 * TOTC], i16, kind="ExternalInput")
    dibh = nc.dram_tensor("dib", [128, 8 * TOTC], i16, kind="ExternalInput")
    outh = nc.dram_tensor("logits", [128, TOTC], f32, kind="ExternalOutput")

    tsh = [nc.dram_tensor(f"tsh{l}", [SP, WG if l < 4 else WZ], f16,
                          kind="Internal") for l in (1, 2, 3, 4)]
    tab = [nc.dram_tensor(f"tab{l}", [G, WG if l < 4 else WZ], f16,
                          kind="Internal", addr_space="Shared")
           for l in (1, 2, 3, 4)]
    zshh = nc.dram_tensor("zsh", [SP, WZ], f16, kind="Internal")
    ztab = nc.dram_tensor("ztab", [G, WZ], f16, kind="Internal",
                          addr_space="Shared")

    off0 = np.concatenate([[0], np.cumsum(K0)]).astype(int)
    off1 = np.concatenate([[0], np.cumsum(K1)]).astype(int)
    offm = np.concatenate([[0], np.cumsum(np.array(K0) + np.array(K1))]
                          ).astype(int)

    from concourse.masks import make_identity

    def copy8(dram_src, dst_handle, t, W):
        """sim AllGather: one DMA replicating a 128-row tile to 8 shard
        slots of the table (stride-0 source)."""
        r0 = t * 128
        in_ap = bass.AP(bass.DRamTensorHandle(dram_src.name,
                                              list(dram_src.shape),
                                              dram_src.dtype),
                        r0 * W, [[0, NCORES], [W, 128], [1, W]])
        out_ap = bass.AP(bass.DRamTensorHandle(dst_handle.name,
                                               list(dst_handle.shape),
                                               dst_handle.dtype),
                         r0 * W, [[SP * W, NCORES], [W, 128], [1, W]])
        nc.sync.dma_start(out=out_ap, in_=in_ap)

    with tile.TileContext(nc) as tc:
        with tc.tile_pool(name="const", bufs=1) as cp, \
             tc.tile_pool(name="psum", bufs=2, space="PSUM") as pp, \
             tc.tile_pool(name="psum2", bufs=2, space="PSUM") as pp2, \
             tc.tile_pool(name="sb", bufs=3) as sb, \
             tc.tile_pool(name="gath", bufs=3) as gp, \
             tc.tile_pool(name="gath2", bufs=2) as gp2, \
             tc.tile_pool(name="dgp", bufs=16) as dgp:

            ident = cp.tile([128, 128], f16, tag="ident")
            make_identity(nc, ident[:])
            ones1 = cp.tile([1, 128], f16, tag="ones1")
            nc.vector.memset(ones1[:], 1.0)

            # resident static streams
            ix0s = cp.tile([128, 8 * SK0], i16, tag="ix0s")
            nc.sync.dma_start(out=ix0s[:], in_=ix0h.ap())
            ix1s = cp.tile([128, 8 * SK1], i16, tag="ix1s")
            nc.sync.dma_start(out=ix1s[:], in_=ix1h.ap())
            msks = cp.tile([128, SKT], f16, tag="msks")
            nc.sync.dma_start(out=msks[:], in_=mskh.ap())
            nwts = cp.tile([128, SKT], f32, tag="nwts")
            nc.sync.dma_start(out=nwts[:], in_=nwth.ap())
            dv2s = cp.tile([128, NT], f32, tag="dv2s")
            nc.sync.dma_start(out=dv2s[:], in_=dv2h.ap())

            wt = []
            for l in (1, 2, 3):
                w = cp.tile([128, WO], f16, tag=f"wx{l}")
                nc.sync.dma_start(out=w[:], in_=wxh[l - 1].ap())
                wt.append(w)
            w4t = cp.tile([128, OUT], f16, tag="w4t")
            nc.sync.dma_start(out=w4t[:], in_=w4h.ap())

            bb = []
            for l in (1, 2, 3, 4):
                wdt = HID if l < 4 else OUT
                bs = sb.tile([1, wdt], f16, tag="bld")
                nc.sync.dma_start(out=bs[:], in_=bih[l - 1].ap())
                bps = pp.tile([128, wdt], f32, tag="bps")
                nc.tensor.matmul(bps[:], lhsT=ones1[:], rhs=bs[:],
                                 start=True, stop=True)
                bt = cp.tile([128, wdt], f16, tag=f"bb{l}")
                nc.vector.tensor_copy(bt[:], bps[:])
                bb.append(bt)

            # two resident node-table buffers (layer parity)
            otb0 = cp.tile([128, NT, WO], f16, tag="otb0")
            otb1 = cp.tile([128, NT, WO], f16, tag="otb1")
            otb = [otb0, otb1]

            rg = [list(range(NCORES))]

            def node_tile(l, t, ht):
                """ht: [128,128] f16 SBUF tile of node features; emits
                projection into resident ot buffer + table shard store."""
                Wp = WO if l < 4 else OUT
                Wrow = WG if l < 4 else WZ
                par = (l - 1) % 2
                ot = otb[par][:, t, :Wp]
                tp = pp.tile([128, 128], f16, tag="tp")
                nc.tensor.transpose(tp[:], ht[:], ident[:])
                hT = sb.tile([128, 128], f16, tag="hT")
                nc.scalar.activation(hT[:], tp[:], Act.Copy)
                mm = pp.tile([128, Wp], f32, tag="mm")
                nc.tensor.matmul(mm[:], lhsT=hT[:],
                                 rhs=(wt[l - 1] if l < 4 else w4t)[:],
                                 start=True, stop=True)
                nc.scalar.activation(ot, mm[:], Act.Copy)
                r0 = t * 128
                nc.sync.dma_start(out=tsh[l - 1].ap()[r0:r0 + 128, 0:Wp],
                                  in_=ot)
                if sim_mode:
                    copy8(tsh[l - 1], tab[l - 1], t, Wrow)

            def edge_tile(l, t):
                """emits edge phase for tile t of layer l; returns the
                output feature tile ([128,128] f16 for GAT, [128,64] GCN)."""
                k0, k1 = K0[t], K1[t]
                kt = k0 + k1
                W = WG if l < 4 else WZ
                par = (l - 1) % 2
                ot = otb[par]
                g0 = gp.tile([128, k0, W], f16, tag="g0")
                nc.gpsimd.dma_gather(
                    out_ap=g0[:], in_ap=tab[l - 1].ap()[0:HALF],
                    idxs_ap=ix0s[:, 8 * off0[t]:8 * (off0[t] + k0)],
                    num_idxs=128 * k0, num_idxs_reg=128 * k0,
                    elem_size=W, single_packet=False)
                g1 = gp2.tile([128, k1, W], f16, tag="g1")
                nc.gpsimd.dma_gather(
                    out_ap=g1[:], in_ap=tab[l - 1].ap()[HALF:G],
                    idxs_ap=ix1s[:, 8 * off1[t]:8 * (off1[t] + k1)],
                    num_idxs=128 * k1, num_idxs_reg=128 * k1,
                    elem_size=W, single_packet=False)

                odim = HID if l < 4 else OUT
                slots = [(g0, k) for k in range(k0)] + \
                        [(g1, k) for k in range(k1)]
                pacc = pp2.tile([128, odim], f32, tag="pacc")

                if l < 4:
                    sc = sb.tile([128, kt], f16, tag="sc")
                    hd = ot[:, t, 129:130]
                    nc.vector.tensor_tensor(
                        out=sc[:, :k0], in0=g0[:, :, 128:129],
                        in1=hd.to_broadcast([128, k0]), op=AluOp.add)
                    nc.vector.tensor_tensor(
                        out=sc[:, k0:], in0=g1[:, :, 128:129],
                        in1=hd.to_broadcast([128, k1]), op=AluOp.add)
                    nc.vector.scalar_tensor_tensor(
                        out=sc[:], in0=sc[:], scalar=NEG, in1=sc[:],
                        op0=AluOp.mult, op1=AluOp.max)
                    nc.vector.scalar_tensor_tensor(
                        out=sc[:], in0=sc[:], scalar=60.0,
                        in1=msks[:, offm[t]:offm[t] + kt],
                        op0=AluOp.min, op1=AluOp.add)
                    p = sb.tile([128, kt], f32, tag="p")
                    ss = sb.tile([128, 1], f32, tag="ss")
                    nc.scalar.activation(p[:], sc[:], Act.Exp,
                                         accum_out=ss[:])
                    # self-loop score
                    scl = sb.tile([128, 1], f16, tag="scl")
                    nc.vector.tensor_tensor(out=scl[:],
                                            in0=ot[:, t, 128:129],
                                            in1=ot[:, t, 129:130],
                                            op=AluOp.add)
                    nc.vector.scalar_tensor_tensor(
                        out=scl[:], in0=scl[:], scalar=NEG, in1=scl[:],
                        op0=AluOp.mult, op1=AluOp.max)
                    psf = sb.tile([128, 1], f32, tag="psf")
                    nc.scalar.activation(psf[:], scl[:], Act.Exp)
                    wcol = lambda j: p[:, j:j + 1]
                else:
                    psf = dv2s[:, t:t + 1]
                    wcol = lambda j: nwts[:, offm[t] + j:offm[t] + j + 1]

                # aggregation: DVE tensor_scalar builds diag(w)*I (4x
                # mode, 93ns), PE matmul accumulates into PSUM fp32.
                # self-loop slot reads the local ot tile.
                for j, (gt, k) in enumerate(slots):
                    dg = dgp.tile([128, 128], f16, tag="dg")
                    nc.vector.tensor_scalar(out=dg[:], in0=ident[:],
                                            scalar1=wcol(j), scalar2=None,
                                            op0=AluOp.mult)
                    nc.tensor.matmul(pacc[:], lhsT=dg[:],
                                     rhs=gt[:, k, :odim],
                                     start=(j == 0), stop=False)
                dg = dgp.tile([128, 128], f16, tag="dg")
                nc.vector.tensor_scalar(out=dg[:], in0=ident[:],
                                        scalar1=psf[:, :1], scalar2=None,
                                        op0=AluOp.mult)
                nc.tensor.matmul(pacc[:], lhsT=dg[:], rhs=ot[:, t, :odim],
                                 start=False, stop=True)

                pc = sb.tile([128, odim], f16, tag="pc")
                nc.scalar.activation(pc[:], pacc[:], Act.Copy)
                hn = sb.tile([128, odim], f16, tag="hn")
                if l < 4:
                    st = sb.tile([128, 1], f32, tag="st")
                    nc.vector.tensor_tensor(out=st[:], in0=ss[:],
                                            in1=psf[:], op=AluOp.add)
                    nc.vector.tensor_scalar_max(st[:], st[:], 1e-30)
                    rr = sb.tile([128, 1], f32, tag="rr")
                    nc.vector.reciprocal(rr[:], st[:])
                    nc.vector.scalar_tensor_tensor(
                        out=hn[:], in0=pc[:], scalar=rr[:, :1],
                        in1=bb[l - 1][:], op0=AluOp.mult, op1=AluOp.add)
                    nc.vector.tensor_scalar_max(hn[:], hn[:], 0.0)
                else:
                    nc.vector.tensor_tensor(out=hn[:], in0=pc[:],
                                            in1=bb[3][:], op=AluOp.add)
                return hn

            # ---- layer 1 node phase ----
            for t in range(NT):
                ht = sb.tile([128, 128], f16, tag="ht")
                nc.sync.dma_start(out=ht[:],
                                  in_=xs.ap()[t * 128:(t + 1) * 128, :])
                node_tile(1, t, ht)
            if not sim_mode:
                nc.gpsimd.collective_compute(
                    "AllGather", AluOp.bypass, replica_groups=rg,
                    ins=[tsh[0].ap()], outs=[tab[0].ap()])

            # ---- layers 1..3 edge + next node, pipelined per tile ----
            for l in (1, 2, 3):
                for t in range(NT):
                    hn = edge_tile(l, t)
                    node_tile(l + 1, t, hn)
                if not sim_mode:
                    nc.gpsimd.collective_compute(
                        "AllGather", AluOp.bypass, replica_groups=rg,
                        ins=[tsh[l].ap()], outs=[tab[l].ap()])

            # ---- GCN edge phase ----
            for t in range(NT):
                zt = edge_tile(4, t)
                r0 = t * 128
                nc.sync.dma_start(out=zshh.ap()[r0:r0 + 128, 0:OUT],
                                  in_=zt[:])
                if sim_mode:
                    copy8(zshh, ztab, t, WZ)
            if not sim_mode:
                nc.gpsimd.collective_compute(
                    "AllGather", AluOp.bypass, replica_groups=rg,
                    ins=[zshh.ap()], outs=[ztab.ap()])

            # ---- decode (variable-size batches) ----
            co = 0
            for g, s in DB:
                baseA = HALF * (g >> 1)
                baseB = HALF * (g & 1)
                pbc = s // 128
                ia = sb.tile([128, 8 * PBC], i16, tag="ia")
                nc.sync.dma_start(
                    out=ia[:, :8 * pbc],
                    in_=diah.ap()[:, 8 * co:8 * (co + pbc)])
                ib = sb.tile([128, 8 * PBC], i16, tag="ib")
                nc.sync.dma_start(
                    out=ib[:, :8 * pbc],
                    in_=dibh.ap()[:, 8 * co:8 * (co + pbc)])
                ga = gp.tile([128, PBC, WZ], f16, tag="g0")
                nc.gpsimd.dma_gather(
                    out_ap=ga[:, :pbc, :], in_ap=ztab.ap()[baseA:baseA + HALF],
                    idxs_ap=ia[:, :8 * pbc], num_idxs=s, num_idxs_reg=s,
                    elem_size=WZ, single_packet=False)
                gb = gp2.tile([128, PBC, WZ], f16, tag="g1")
                nc.gpsimd.dma_gather(
                    out_ap=gb[:, :pbc, :], in_ap=ztab.ap()[baseB:baseB + HALF],
                    idxs_ap=ib[:, :8 * pbc], num_idxs=s, num_idxs_reg=s,
                    elem_size=WZ, single_packet=False)
                pr = sb.tile([128, PBC, OUT], f16, tag="pr")
                nc.vector.tensor_tensor(out=pr[:, :pbc, :],
                                        in0=ga[:, :pbc, :OUT],
                                        in1=gb[:, :pbc, :OUT],
                                        op=mybir.AluOpType.mult)
                dt_ = sb.tile([128, PBC], f32, tag="dt")
                nc.vector.tensor_reduce(dt_[:, :pbc], pr[:, :pbc, :],
                                        axis=mybir.AxisListType.X,
                                        op=mybir.AluOpType.add)
                nc.sync.dma_start(out=outh.ap()[:, co:co + pbc],
                                  in_=dt_[:, :pbc])
                co += pbc

    nc.compile()
    return nc


def kernel(**inputs):
    prof, in_maps, unshard, meta = _prep(
        inputs["x"], inputs["edge_index"], inputs["edge_label_index"],
        inputs["W1"], inputs["a1s"], inputs["a1d"], inputs["b1"],
        inputs["W2"], inputs["a2s"], inputs["a2d"], inputs["b2"],
        inputs["W3"], inputs["a3s"], inputs["a3d"], inputs["b3"],
        inputs["W4"], inputs["b4"])
    nc = _build(prof)
    res = bass_utils.run_bass_kernel_spmd(
        nc, in_maps, core_ids=list(range(NCORES)))
    results = res.results

    npc = meta["npc"]
    DB = prof["DB"]
    gcounts = meta["gcounts"]
    out = np.empty(NL, np.float32)
    for c in range(NCORES):
        arr = results[c]["logits"]          # [128, TOTC]
        # flat slot j within a group = cc*128 + p -> arr[p, co+cc]
        flats = {g: [] for g in range(4)}
        co = 0
        for g, s in DB:
            pbc = s // 128
            flats[g].append(arr[:, co:co + pbc].T.reshape(-1))
            co += pbc
        vals = []
        for g in range(4):
            fl = np.concatenate(flats[g])
            vals.append(fl[:gcounts[c][g]])
        sorted_vals = np.concatenate(vals)
        block = np.empty(npc, np.float32)
        block[unshard[c]] = sorted_vals
        out[c * npc:(c + 1) * npc] = block
    return out
